# revision 28
# baseline (speedup 1.0000x reference)
"""Expert-parallel MoE MLP (top-2 of 8 experts) on 8 TRN2 NeuronCores.

Strategy (expert-parallel, per sharding hint):
  - core e holds expert e's weights (w1[e], w2[e], host-pre-transposed, bf16)
  - host dispatches tokens by expert id: core e receives the tokens routed to
    expert e in a block-padded layout split into sub-buffers (see _sub_bounds);
    sub-buffer g holds one slot-range of every (expert->owner) block, so the
    AllToAll splits into chunks that fire as compute progresses (symmetric
    halves measured best: 3 asymmetric chunks paid more in entry floors than
    the earlier firing saved)
  - core e computes y_part = [silu(x_e @ w1[e]^T) * c_e] @ w2[e]^T for its
    tokens (bf16 matmuls, fp32 accumulate, combine weights applied in fp32,
    partials exchanged in bf16)
  - G chunked AllToAlls return each owner core the partial rows for its 512
    tokens; the owner gathers the (up to) 2 partial rows per token, adds
    them, and writes its [512, 1024] fp32 output shard
  - host concatenates the 8 output shards
"""

import sys

sys.path.insert(0, "/opt/trn_rl_repo")

import numpy as np
import ml_dtypes

import concourse.bass as bass
import concourse.tile as tile
from concourse import bacc, mybir
from concourse.bass_utils import run_bass_kernel_spmd

S, DM, DF, E, TOPK = 4096, 1024, 2048, 8, 2
NCORES = 8
P = 128
OWN = S // NCORES  # tokens per owner core
G = 2  # number of chunked AllToAlls
MM1_STATIONARY = True  # reuse w1 as stationary across chunks (1 ldw : n_chunk mms)

_PROGRAM_CACHE: dict = {}


def _sub_bounds(blk: int) -> list[int]:
    """Slot-range boundaries for the chunked A2As (asymmetric: big chunk
    first so most of the exchange fires early in the compute window)."""
    half = blk // 2
    return [0, half, blk] if 0 < half < blk else [0, blk]


def _chunks_of(tokpad: int) -> list[tuple[int, int]]:
    """Split tokpad into (start, size) chunks, each a multiple of 128, <= 512."""
    n_ch = -(-tokpad // 512)
    base = tokpad // n_ch // P * P
    sizes = [base] * n_ch
    rem = tokpad - base * n_ch
    i = 0
    while rem > 0:
        sizes[i] += P
        rem -= P
        i = (i + 1) % n_ch
    out, pos = [], 0
    for s in sizes:
        out.append((pos, s))
        pos += s
    assert pos == tokpad
    return out


def _emit(nc, tc, ctx, tokpad: int, reps: int, mode: str = "full"):
    dt = mybir.dt
    ntokm = tokpad // P
    chunks = _chunks_of(tokpad)
    bounds = _sub_bounds(tokpad // NCORES)  # slot boundaries per block
    row_bounds = [NCORES * b for b in bounds]  # sendbuf row boundaries
    n_a2a = len(bounds) - 1

    xT = nc.dram_tensor("xT", [DM, tokpad], dt.bfloat16, kind="ExternalInput").ap()
    w1t = nc.dram_tensor("w1t", [DM, DF], dt.bfloat16, kind="ExternalInput").ap()
    w2t = nc.dram_tensor("w2t", [DF, DM], dt.bfloat16, kind="ExternalInput").ap()
    cv = nc.dram_tensor("cv", [tokpad], dt.float32, kind="ExternalInput").ap()
    g0 = nc.dram_tensor("g0", [OWN], dt.int32, kind="ExternalInput").ap()
    g1 = nc.dram_tensor("g1", [OWN], dt.int32, kind="ExternalInput").ap()
    g0b = nc.dram_tensor("g0b", [OWN], dt.int32, kind="ExternalInput").ap()
    g1b = nc.dram_tensor("g1b", [OWN], dt.int32, kind="ExternalInput").ap()
    yout = nc.dram_tensor("yout", [OWN, DM], dt.float32, kind="ExternalOutput").ap()
    sendbuf = nc.dram_tensor("sendbuf", [tokpad, DM], dt.bfloat16).ap()
    recvbuf = nc.dram_tensor("recvbuf", [tokpad + 1, DM], dt.bfloat16).ap()

    n_chunks = len(chunks)
    wpool = ctx.enter_context(tc.tile_pool(name="w", bufs=1))
    hpool = ctx.enter_context(
        tc.tile_pool(name="h", bufs=(DF // P) * n_chunks + 2 if MM1_STATIONARY else 20)
    )
    ypool = ctx.enter_context(tc.tile_pool(name="y", bufs=3))
    gpool = ctx.enter_context(tc.tile_pool(name="g", bufs=4))
    phpool = ctx.enter_context(
        tc.tile_pool(name="ph", bufs=n_chunks + 1 if MM1_STATIONARY else 2, space="PSUM")
    )
    pypool = ctx.enter_context(tc.tile_pool(name="py", bufs=4, space="PSUM"))

    for _rep in range(reps):
        # ---- loads: w1/x interleaved (first matmuls need them), then the rest
        w1sb = wpool.tile([P, DM // P, DF], dt.bfloat16, tag="w1sb")
        w1r = w1t.rearrange("(o p) f -> p o f", p=P)
        xsb = wpool.tile([P, DM // P, tokpad], dt.bfloat16, tag="xsb")
        xr = xT.rearrange("(o p) t -> p o t", p=P)
        for k in range(DM // P):
            nc.sync.dma_start(w1sb[:, k, :], w1r[:, k, :])
            nc.sync.dma_start(xsb[:, k, :], xr[:, k, :])
        csb = wpool.tile([P, ntokm], dt.float32, tag="csb")
        nc.sync.dma_start(csb[:], cv.rearrange("(t p) -> p t", p=P))
        w2sb = wpool.tile([P, DF // P, DM], dt.bfloat16, tag="w2sb")
        w2r = w2t.rearrange("(o p) d -> p o d", p=P)
        for f in range(DF // P):
            nc.sync.dma_start(w2sb[:, f, :], w2r[:, f, :])
        g0sb = wpool.tile([P, OWN // P], dt.int32, tag="g0sb")
        nc.sync.dma_start(g0sb[:], g0.rearrange("(t p) -> p t", p=P))
        g1sb = wpool.tile([P, OWN // P], dt.int32, tag="g1sb")
        nc.sync.dma_start(g1sb[:], g1.rearrange("(t p) -> p t", p=P))
        g0bsb = wpool.tile([P, OWN // P], dt.int32, tag="g0bsb")
        nc.sync.dma_start(g0bsb[:], g0b.rearrange("(t p) -> p t", p=P))
        g1bsb = wpool.tile([P, OWN // P], dt.int32, tag="g1bsb")
        nc.sync.dma_start(g1bsb[:], g1b.rearrange("(t p) -> p t", p=P))
        zrow = wpool.tile([1, DM], dt.bfloat16, tag="zrow")
        nc.vector.memset(zrow[:], 0.0)
        nc.sync.dma_start(recvbuf[tokpad : tokpad + 1, :], zrow[:])

        # ---- expert MLP over token chunks; fire A2A chunk g once its rows exist
        a2a_fired = 0

        def fire_a2a(upto_row):
            nonlocal a2a_fired
            while (
                mode != "compute"
                and a2a_fired < n_a2a
                and row_bounds[a2a_fired + 1] <= upto_row
            ):
                r0, r1 = row_bounds[a2a_fired], row_bounds[a2a_fired + 1]
                nc.gpsimd.collective_compute(
                    "AllToAll",
                    mybir.AluOpType.bypass,
                    replica_groups=[list(range(NCORES))],
                    ins=[sendbuf[r0:r1, :]],
                    outs=[recvbuf[r0:r1, :]],
                )
                a2a_fired += 1

        def mm2_tile(hs_j, c0, csz, tm):  # token m-tile: y = (c * H^T) . w2
            tglob = c0 // P + tm
            py0 = pypool.tile([P, 512], dt.float32, tag="py")
            py1 = pypool.tile([P, 512], dt.float32, tag="py")
            for f in range(DF // P):
                lhs = hs_j[f][:, tm * P : (tm + 1) * P]
                nc.tensor.matmul(
                    py0[:], lhsT=lhs, rhs=w2sb[:, f, 0:512],
                    start=(f == 0), stop=(f == DF // P - 1),
                )
                nc.tensor.matmul(
                    py1[:], lhsT=lhs, rhs=w2sb[:, f, 512:1024],
                    start=(f == 0), stop=(f == DF // P - 1),
                )
            y_sb = ypool.tile([P, DM], dt.bfloat16, tag="y")
            nc.vector.tensor_scalar_mul(
                y_sb[:, 0:512], py0[:], csb[:, tglob : tglob + 1]
            )
            nc.vector.tensor_scalar_mul(
                y_sb[:, 512:1024], py1[:], csb[:, tglob : tglob + 1]
            )
            nc.sync.dma_start(sendbuf[tglob * P : (tglob + 1) * P, :], y_sb[:])
            fire_a2a((tglob + 1) * P)

        def silu_to_h(ph, csz):
            h_i = hpool.tile([P, csz], dt.bfloat16, tag="h")
            nc.scalar.activation(h_i[:], ph[:], mybir.ActivationFunctionType.Silu)
            return h_i

        if mode != "a2a":
            if MM1_STATIONARY:
                hs_all = [[None] * (DF // P) for _ in chunks]
                for i in range(DF // P):
                    phs = [
                        phpool.tile([P, csz], dt.float32, tag="ph", name=f"ph{j}")
                        for j, (_c0, csz) in enumerate(chunks)
                    ]
                    for k in range(DM // P):
                        for j, (c0, csz) in enumerate(chunks):
                            nc.tensor.matmul(
                                phs[j][:],
                                lhsT=w1sb[:, k, i * P : (i + 1) * P],
                                rhs=xsb[:, k, c0 : c0 + csz],
                                start=(k == 0),
                                stop=(k == DM // P - 1),
                            )
                    for j, (_c0, csz) in enumerate(chunks):
                        hs_all[j][i] = silu_to_h(phs[j], csz)
                for j, (c0, csz) in enumerate(chunks):
                    for tm in range(csz // P):
                        mm2_tile(hs_all[j], c0, csz, tm)
            else:
                for c0, csz in chunks:
                    hs = []
                    for i in range(DF // P):  # f-tiles: H[f] = silu(w1 . x)
                        ph = phpool.tile([P, csz], dt.float32, tag="ph")
                        for k in range(DM // P):
                            nc.tensor.matmul(
                                ph[:],
                                lhsT=w1sb[:, k, i * P : (i + 1) * P],
                                rhs=xsb[:, k, c0 : c0 + csz],
                                start=(k == 0),
                                stop=(k == DM // P - 1),
                            )
                        hs.append(silu_to_h(ph, csz))
                    for tm in range(csz // P):
                        mm2_tile(hs, c0, csz, tm)
            fire_a2a(tokpad)
        else:
            fire_a2a(tokpad)

        # ---- combine: per owned token, add its (up to) 2 partial rows.
        # Two-pass gathers: pass 1 reads only recvbuf[:split] (rows delivered
        # by A2A chunk 0, so it overlaps the in-flight chunk 1; rows >= split
        # are skipped via bounds_check), pass 2 fills the rest from the
        # sentinel-masked index arrays after chunk 1 lands.
        split = row_bounds[1] if n_a2a > 1 else 0
        for tm in range(OWN // P):
            ga = gpool.tile([P, DM], dt.bfloat16, tag="ga")
            if split:
                nc.gpsimd.indirect_dma_start(
                    out=ga[:],
                    out_offset=None,
                    in_=recvbuf[:split, :],
                    in_offset=bass.IndirectOffsetOnAxis(
                        ap=g0sb[:, tm : tm + 1], axis=0
                    ),
                    bounds_check=split - 1,
                    oob_is_err=False,
                )
            nc.gpsimd.indirect_dma_start(
                out=ga[:],
                out_offset=None,
                in_=recvbuf[:],
                in_offset=bass.IndirectOffsetOnAxis(
                    ap=(g0bsb if split else g0sb)[:, tm : tm + 1], axis=0
                ),
                bounds_check=tokpad if split else None,
                oob_is_err=False if split else True,
            )
            gb = gpool.tile([P, DM], dt.bfloat16, tag="gb")
            if split:
                nc.gpsimd.indirect_dma_start(
                    out=gb[:],
                    out_offset=None,
                    in_=recvbuf[:split, :],
                    in_offset=bass.IndirectOffsetOnAxis(
                        ap=g1sb[:, tm : tm + 1], axis=0
                    ),
                    bounds_check=split - 1,
                    oob_is_err=False,
                )
            nc.gpsimd.indirect_dma_start(
                out=gb[:],
                out_offset=None,
                in_=recvbuf[:],
                in_offset=bass.IndirectOffsetOnAxis(
                    ap=(g1bsb if split else g1sb)[:, tm : tm + 1], axis=0
                ),
                bounds_check=tokpad if split else None,
                oob_is_err=False if split else True,
            )
            ys = gpool.tile([P, DM], dt.float32, tag="ys")
            nc.vector.tensor_add(ys[:], ga[:], gb[:])
            nc.sync.dma_start(yout[tm * P : (tm + 1) * P, :], ys[:])


def _build_program(tokpad: int, reps: int = 1, mode: str = "full"):
    key = (tokpad, reps, mode, "v4", MM1_STATIONARY)
    if key in _PROGRAM_CACHE:
        return _PROGRAM_CACHE[key]
    from contextlib import ExitStack

    nc = bacc.Bacc(
        "TRN2",
        target_bir_lowering=False,
        debug=False,
        enable_asserts=True,
        num_devices=NCORES,
    )
    with tile.TileContext(nc) as tc:
        with ExitStack() as ctx:
            _emit(nc, tc, ctx, tokpad, reps, mode)
    nc.compile()
    _PROGRAM_CACHE[key] = nc
    return nc


def _prepare(x, topk_e, topk_w):
    """Host-side routing: dispatch tokens to experts.

    Column/sendbuf-row layout on core e (tokpad = 8*BLK rows, G sub-buffers):
      token with slot s in (expert e -> owner d) block lives at row
        (s // SUBBLK) * (8*SUBBLK) + d * SUBBLK + (s % SUBBLK)
    so rows [g*tokpad/G, (g+1)*tokpad/G) form A2A chunk g = slot-range
    [g*SUBBLK, (g+1)*SUBBLK) of all 8 destination blocks.
    """
    bf16 = ml_dtypes.bfloat16
    c = np.zeros((S, E), dtype=np.float32)
    np.add.at(c, (np.arange(S)[:, None], topk_e), topk_w.astype(np.float32))

    toks = [np.nonzero((topk_e == e).any(axis=1))[0] for e in range(E)]
    cnt = np.zeros((E, NCORES), dtype=np.int64)
    for e in range(E):
        d = toks[e] // OWN
        for dd in range(NCORES):
            cnt[e, dd] = int((d == dd).sum())
    blk = int(-(-cnt.max() // 16) * 16)  # multiple of 16 (so tokpad % 128 == 0)
    tokpad = blk * NCORES
    zero_row = tokpad
    bounds = _sub_bounds(blk)

    def row_of(d, s):
        for g in range(len(bounds) - 1):
            if s < bounds[g + 1]:
                sz = bounds[g + 1] - bounds[g]
                return NCORES * bounds[g] + d * sz + (s - bounds[g])
        raise AssertionError(s)

    in_maps = []
    slot_of = {}
    for e in range(E):
        te = toks[e]
        d = te // OWN
        seg_start = np.searchsorted(te, np.arange(NCORES) * OWN)
        slots = np.arange(len(te)) - seg_start[d]
        col = np.array([row_of(dd, ss) for dd, ss in zip(d, slots)], dtype=np.int64)
        for t, sl in zip(te, slots):
            slot_of[(e, int(t))] = int(sl)
        xT_e = np.zeros((DM, tokpad), dtype=bf16)
        if len(te):
            xT_e[:, col] = x[te].T.astype(bf16)
        cv_e = np.zeros(tokpad, dtype=np.float32)
        cv_e[col] = c[te, e]
        in_maps.append({"xT": xT_e, "cv": cv_e})

    for d in range(NCORES):
        g0a = np.full(OWN, zero_row, dtype=np.int32)
        g1a = np.full(OWN, zero_row, dtype=np.int32)
        for t_loc in range(OWN):
            t = d * OWN + t_loc
            es = np.unique(topk_e[t])
            g0a[t_loc] = row_of(int(es[0]), slot_of[(int(es[0]), t)])
            if len(es) > 1:
                g1a[t_loc] = row_of(int(es[1]), slot_of[(int(es[1]), t)])
        in_maps[d]["g0"] = g0a
        in_maps[d]["g1"] = g1a
        # pass-2 indices: rows already fetched in pass 1 become a skipped
        # sentinel (> tokpad bounds_check); rows >= split (incl. zero row)
        # stay for pass 2
        split = NCORES * bounds[1] if len(bounds) > 2 else 0
        skip = np.int32(tokpad + 1)
        in_maps[d]["g0b"] = np.where(g0a >= split, g0a, skip).astype(np.int32)
        in_maps[d]["g1b"] = np.where(g1a >= split, g1a, skip).astype(np.int32)

    return in_maps, tokpad


def prepare_in_maps(x, topk_e, topk_w, w1, w2):
    bf16 = ml_dtypes.bfloat16
    in_maps, tokpad = _prepare(
        np.asarray(x), np.asarray(topk_e), np.asarray(topk_w)
    )
    for e in range(E):
        in_maps[e]["w1t"] = np.ascontiguousarray(np.asarray(w1)[e].T).astype(bf16)
        in_maps[e]["w2t"] = np.ascontiguousarray(np.asarray(w2)[e].T).astype(bf16)
    return in_maps, tokpad


def kernel(x, topk_e, topk_w, w1, w2):
    in_maps, tokpad = prepare_in_maps(x, topk_e, topk_w, w1, w2)
    nc = _build_program(tokpad)
    res = run_bass_kernel_spmd(nc, in_maps, list(range(NCORES)))
    out = np.concatenate(
        [res.results[d]["yout"] for d in range(NCORES)], axis=0
    )
    return out.astype(np.float32)


# revision 31
# speedup vs baseline: 1.0439x; 1.0439x over previous
"""Expert-parallel MoE MLP (top-2 of 8 experts) on 8 TRN2 NeuronCores.

Strategy (expert-parallel, per sharding hint):
  - core e holds expert e's weights (w1[e], w2[e], host-pre-transposed, bf16)
  - host dispatches tokens by expert id: core e receives the tokens routed to
    expert e in a block-padded layout split into sub-buffers (see _sub_bounds);
    sub-buffer g holds one slot-range of every (expert->owner) block, so the
    AllToAll splits into chunks that fire as compute progresses (symmetric
    halves measured best: 3 asymmetric chunks paid more in entry floors than
    the earlier firing saved)
  - core e computes y_part = [silu(x_e @ w1[e]^T) * c_e] @ w2[e]^T for its
    tokens (bf16 matmuls, fp32 accumulate, combine weights applied in fp32,
    partials exchanged in bf16)
  - G chunked AllToAlls return each owner core the partial rows for its 512
    tokens; the owner gathers the (up to) 2 partial rows per token, adds
    them, and writes its [512, 1024] fp32 output shard
  - host concatenates the 8 output shards
"""

import sys

sys.path.insert(0, "/opt/trn_rl_repo")

import numpy as np
import ml_dtypes

import concourse.bass as bass
import concourse.tile as tile
from concourse import bacc, mybir
from concourse.bass_utils import run_bass_kernel_spmd

S, DM, DF, E, TOPK = 4096, 1024, 2048, 8, 2
NCORES = 8
P = 128
OWN = S // NCORES  # tokens per owner core
G = 2  # number of chunked AllToAlls
MM1_STATIONARY = True  # reuse w1 as stationary across chunks (1 ldw : n_chunk mms)

_PROGRAM_CACHE: dict = {}


def _sub_bounds(blk: int) -> list[int]:
    """Slot-range boundaries for the chunked A2As. The split lands on an
    m-tile row boundary (8*b1 % 128 == 0 when b1 % 16 == 0) just past the
    midpoint: chunk 0 fires at the same compute tile as a 50/50 split would,
    but the exposed final chunk is ~10% smaller."""
    b1 = max(16, (blk * 5 // 9) // 16 * 16)
    return [0, b1, blk] if 0 < b1 < blk else [0, blk]


def _chunks_of(tokpad: int) -> list[tuple[int, int]]:
    """Split tokpad into (start, size) chunks, each a multiple of 128, <= 512."""
    n_ch = -(-tokpad // 512)
    base = tokpad // n_ch // P * P
    sizes = [base] * n_ch
    rem = tokpad - base * n_ch
    i = 0
    while rem > 0:
        sizes[i] += P
        rem -= P
        i = (i + 1) % n_ch
    out, pos = [], 0
    for s in sizes:
        out.append((pos, s))
        pos += s
    assert pos == tokpad
    return out


def _emit(nc, tc, ctx, tokpad: int, reps: int, mode: str = "full"):
    dt = mybir.dt
    ntokm = tokpad // P
    chunks = _chunks_of(tokpad)
    bounds = _sub_bounds(tokpad // NCORES)  # slot boundaries per block
    row_bounds = [NCORES * b for b in bounds]  # sendbuf row boundaries
    n_a2a = len(bounds) - 1

    xT = nc.dram_tensor("xT", [DM, tokpad], dt.bfloat16, kind="ExternalInput").ap()
    w1t = nc.dram_tensor("w1t", [DM, DF], dt.bfloat16, kind="ExternalInput").ap()
    w2t = nc.dram_tensor("w2t", [DF, DM], dt.bfloat16, kind="ExternalInput").ap()
    cv = nc.dram_tensor("cv", [tokpad], dt.float32, kind="ExternalInput").ap()
    g0 = nc.dram_tensor("g0", [OWN], dt.int32, kind="ExternalInput").ap()
    g1 = nc.dram_tensor("g1", [OWN], dt.int32, kind="ExternalInput").ap()
    yout = nc.dram_tensor("yout", [OWN, DM], dt.float32, kind="ExternalOutput").ap()
    sendbuf = nc.dram_tensor("sendbuf", [tokpad, DM], dt.bfloat16).ap()
    recvbuf = nc.dram_tensor("recvbuf", [tokpad + 1, DM], dt.bfloat16).ap()

    n_chunks = len(chunks)
    wpool = ctx.enter_context(tc.tile_pool(name="w", bufs=1))
    hpool = ctx.enter_context(
        tc.tile_pool(name="h", bufs=(DF // P) * n_chunks + 2 if MM1_STATIONARY else 20)
    )
    ypool = ctx.enter_context(tc.tile_pool(name="y", bufs=4))
    gpool = ctx.enter_context(tc.tile_pool(name="g", bufs=4))
    phpool = ctx.enter_context(
        tc.tile_pool(name="ph", bufs=n_chunks + 1 if MM1_STATIONARY else 2, space="PSUM")
    )
    pypool = ctx.enter_context(tc.tile_pool(name="py", bufs=4, space="PSUM"))

    for _rep in range(reps):
        # ---- loads: w1/x interleaved (first matmuls need them), then the rest
        w1sb = wpool.tile([P, DM // P, DF], dt.bfloat16, tag="w1sb")
        w1r = w1t.rearrange("(o p) f -> p o f", p=P)
        xsb = wpool.tile([P, DM // P, tokpad], dt.bfloat16, tag="xsb")
        xr = xT.rearrange("(o p) t -> p o t", p=P)
        for k in range(DM // P):
            nc.sync.dma_start(w1sb[:, k, :], w1r[:, k, :])
            nc.sync.dma_start(xsb[:, k, :], xr[:, k, :])
        csb = wpool.tile([P, ntokm], dt.float32, tag="csb")
        nc.sync.dma_start(csb[:], cv.rearrange("(t p) -> p t", p=P))
        w2sb = wpool.tile([P, DF // P, DM], dt.bfloat16, tag="w2sb")
        w2r = w2t.rearrange("(o p) d -> p o d", p=P)
        for f in range(DF // P):
            nc.sync.dma_start(w2sb[:, f, :], w2r[:, f, :])
        g0sb = wpool.tile([P, OWN // P], dt.int32, tag="g0sb")
        nc.sync.dma_start(g0sb[:], g0.rearrange("(t p) -> p t", p=P))
        g1sb = wpool.tile([P, OWN // P], dt.int32, tag="g1sb")
        nc.sync.dma_start(g1sb[:], g1.rearrange("(t p) -> p t", p=P))
        zrow = wpool.tile([1, DM], dt.bfloat16, tag="zrow")
        nc.vector.memset(zrow[:], 0.0)
        nc.sync.dma_start(recvbuf[tokpad : tokpad + 1, :], zrow[:])

        # ---- expert MLP over token chunks; fire A2A chunk g once its rows exist
        a2a_fired = 0

        def fire_a2a(upto_row):
            nonlocal a2a_fired
            while (
                mode != "compute"
                and a2a_fired < n_a2a
                and row_bounds[a2a_fired + 1] <= upto_row
            ):
                r0, r1 = row_bounds[a2a_fired], row_bounds[a2a_fired + 1]
                nc.gpsimd.collective_compute(
                    "AllToAll",
                    mybir.AluOpType.bypass,
                    replica_groups=[list(range(NCORES))],
                    ins=[sendbuf[r0:r1, :]],
                    outs=[recvbuf[r0:r1, :]],
                )
                a2a_fired += 1

        def mm2_tile(hs_j, c0, csz, tm):  # token m-tile: y = (c * H^T) . w2
            tglob = c0 // P + tm
            py0 = pypool.tile([P, 512], dt.float32, tag="py")
            py1 = pypool.tile([P, 512], dt.float32, tag="py")
            for f in range(DF // P):
                lhs = hs_j[f][:, tm * P : (tm + 1) * P]
                nc.tensor.matmul(
                    py0[:], lhsT=lhs, rhs=w2sb[:, f, 0:512],
                    start=(f == 0), stop=(f == DF // P - 1),
                )
                nc.tensor.matmul(
                    py1[:], lhsT=lhs, rhs=w2sb[:, f, 512:1024],
                    start=(f == 0), stop=(f == DF // P - 1),
                )
            y_sb = ypool.tile([P, DM], dt.bfloat16, tag="y")
            nc.vector.tensor_scalar_mul(
                y_sb[:, 0:512], py0[:], csb[:, tglob : tglob + 1]
            )
            nc.vector.tensor_scalar_mul(
                y_sb[:, 512:1024], py1[:], csb[:, tglob : tglob + 1]
            )
            nc.sync.dma_start(sendbuf[tglob * P : (tglob + 1) * P, :], y_sb[:])
            fire_a2a((tglob + 1) * P)

        def silu_to_h(ph, csz):
            h_i = hpool.tile([P, csz], dt.bfloat16, tag="h")
            nc.scalar.activation(h_i[:], ph[:], mybir.ActivationFunctionType.Silu)
            return h_i

        if mode != "a2a":
            if MM1_STATIONARY:
                hs_all = [[None] * (DF // P) for _ in chunks]
                for i in range(DF // P):
                    phs = [
                        phpool.tile([P, csz], dt.float32, tag="ph", name=f"ph{j}")
                        for j, (_c0, csz) in enumerate(chunks)
                    ]
                    for k in range(DM // P):
                        for j, (c0, csz) in enumerate(chunks):
                            nc.tensor.matmul(
                                phs[j][:],
                                lhsT=w1sb[:, k, i * P : (i + 1) * P],
                                rhs=xsb[:, k, c0 : c0 + csz],
                                start=(k == 0),
                                stop=(k == DM // P - 1),
                            )
                    for j, (_c0, csz) in enumerate(chunks):
                        hs_all[j][i] = silu_to_h(phs[j], csz)
                for j, (c0, csz) in enumerate(chunks):
                    for tm in range(csz // P):
                        mm2_tile(hs_all[j], c0, csz, tm)
            else:
                for c0, csz in chunks:
                    hs = []
                    for i in range(DF // P):  # f-tiles: H[f] = silu(w1 . x)
                        ph = phpool.tile([P, csz], dt.float32, tag="ph")
                        for k in range(DM // P):
                            nc.tensor.matmul(
                                ph[:],
                                lhsT=w1sb[:, k, i * P : (i + 1) * P],
                                rhs=xsb[:, k, c0 : c0 + csz],
                                start=(k == 0),
                                stop=(k == DM // P - 1),
                            )
                        hs.append(silu_to_h(ph, csz))
                    for tm in range(csz // P):
                        mm2_tile(hs, c0, csz, tm)
            fire_a2a(tokpad)
        else:
            fire_a2a(tokpad)

        # ---- combine: per owned token, add its (up to) 2 partial rows ----
        for tm in range(OWN // P):
            ga = gpool.tile([P, DM], dt.bfloat16, tag="ga")
            nc.gpsimd.indirect_dma_start(
                out=ga[:],
                out_offset=None,
                in_=recvbuf[:],
                in_offset=bass.IndirectOffsetOnAxis(ap=g0sb[:, tm : tm + 1], axis=0),
            )
            gb = gpool.tile([P, DM], dt.bfloat16, tag="gb")
            nc.gpsimd.indirect_dma_start(
                out=gb[:],
                out_offset=None,
                in_=recvbuf[:],
                in_offset=bass.IndirectOffsetOnAxis(ap=g1sb[:, tm : tm + 1], axis=0),
            )
            ys = gpool.tile([P, DM], dt.float32, tag="ys")
            nc.vector.tensor_add(ys[:], ga[:], gb[:])
            nc.sync.dma_start(yout[tm * P : (tm + 1) * P, :], ys[:])


def _build_program(tokpad: int, reps: int = 1, mode: str = "full"):
    key = (tokpad, reps, mode, "v4", MM1_STATIONARY)
    if key in _PROGRAM_CACHE:
        return _PROGRAM_CACHE[key]
    from contextlib import ExitStack

    nc = bacc.Bacc(
        "TRN2",
        target_bir_lowering=False,
        debug=False,
        enable_asserts=True,
        num_devices=NCORES,
    )
    with tile.TileContext(nc) as tc:
        with ExitStack() as ctx:
            _emit(nc, tc, ctx, tokpad, reps, mode)
    nc.compile()
    _PROGRAM_CACHE[key] = nc
    return nc


def _prepare(x, topk_e, topk_w):
    """Host-side routing: dispatch tokens to experts.

    Column/sendbuf-row layout on core e (tokpad = 8*BLK rows, G sub-buffers):
      token with slot s in (expert e -> owner d) block lives at row
        (s // SUBBLK) * (8*SUBBLK) + d * SUBBLK + (s % SUBBLK)
    so rows [g*tokpad/G, (g+1)*tokpad/G) form A2A chunk g = slot-range
    [g*SUBBLK, (g+1)*SUBBLK) of all 8 destination blocks.
    """
    bf16 = ml_dtypes.bfloat16
    c = np.zeros((S, E), dtype=np.float32)
    np.add.at(c, (np.arange(S)[:, None], topk_e), topk_w.astype(np.float32))

    toks = [np.nonzero((topk_e == e).any(axis=1))[0] for e in range(E)]
    cnt = np.zeros((E, NCORES), dtype=np.int64)
    for e in range(E):
        d = toks[e] // OWN
        for dd in range(NCORES):
            cnt[e, dd] = int((d == dd).sum())
    blk = int(-(-cnt.max() // 16) * 16)  # multiple of 16 (so tokpad % 128 == 0)
    tokpad = blk * NCORES
    zero_row = tokpad
    bounds = _sub_bounds(blk)

    def row_of(d, s):
        for g in range(len(bounds) - 1):
            if s < bounds[g + 1]:
                sz = bounds[g + 1] - bounds[g]
                return NCORES * bounds[g] + d * sz + (s - bounds[g])
        raise AssertionError(s)

    in_maps = []
    slot_of = {}
    for e in range(E):
        te = toks[e]
        d = te // OWN
        seg_start = np.searchsorted(te, np.arange(NCORES) * OWN)
        slots = np.arange(len(te)) - seg_start[d]
        col = np.array([row_of(dd, ss) for dd, ss in zip(d, slots)], dtype=np.int64)
        for t, sl in zip(te, slots):
            slot_of[(e, int(t))] = int(sl)
        xT_e = np.zeros((DM, tokpad), dtype=bf16)
        if len(te):
            xT_e[:, col] = x[te].T.astype(bf16)
        cv_e = np.zeros(tokpad, dtype=np.float32)
        cv_e[col] = c[te, e]
        in_maps.append({"xT": xT_e, "cv": cv_e})

    for d in range(NCORES):
        g0a = np.full(OWN, zero_row, dtype=np.int32)
        g1a = np.full(OWN, zero_row, dtype=np.int32)
        for t_loc in range(OWN):
            t = d * OWN + t_loc
            es = np.unique(topk_e[t])
            g0a[t_loc] = row_of(int(es[0]), slot_of[(int(es[0]), t)])
            if len(es) > 1:
                g1a[t_loc] = row_of(int(es[1]), slot_of[(int(es[1]), t)])
        in_maps[d]["g0"] = g0a
        in_maps[d]["g1"] = g1a

    return in_maps, tokpad


def prepare_in_maps(x, topk_e, topk_w, w1, w2):
    bf16 = ml_dtypes.bfloat16
    in_maps, tokpad = _prepare(
        np.asarray(x), np.asarray(topk_e), np.asarray(topk_w)
    )
    for e in range(E):
        in_maps[e]["w1t"] = np.ascontiguousarray(np.asarray(w1)[e].T).astype(bf16)
        in_maps[e]["w2t"] = np.ascontiguousarray(np.asarray(w2)[e].T).astype(bf16)
    return in_maps, tokpad


def kernel(x, topk_e, topk_w, w1, w2):
    in_maps, tokpad = prepare_in_maps(x, topk_e, topk_w, w1, w2)
    nc = _build_program(tokpad)
    res = run_bass_kernel_spmd(nc, in_maps, list(range(NCORES)))
    out = np.concatenate(
        [res.results[d]["yout"] for d in range(NCORES)], axis=0
    )
    return out.astype(np.float32)
